# revision 26
# baseline (speedup 1.0000x reference)
"""Local/global multihead attention on 8 NeuronCores (Trainium2, Bass/Tile).

Sharding: core c = b*2 + hg  (b = batch 0..3, hg = head-group 0/1, 8 heads each).
Each core: q/k/v projections for its 8 heads on its batch, head-local attention,
out-projection restricted to its head-group columns of wo. Host sums the two
head-group partials per batch and adds bo + bv @ wo.T (softmax rows sum to 1).

Key structure (v2):
- Slot-0 head: computed BOTH ways on every core — dense unmasked 2048-key
  softmax (correct for hg0's global head) AND banded (correct for hg1's local
  head); blended by a per-core flag. No gmask input, no mask work in the dense
  path at all.
- k bias dropped entirely (q.bk is constant per query -> softmax invariant).
- Banded scores for a (head, slot) are packed into two [128,768] psum tiles
  (3 windows each) -> one exp + one mask-mul per tile instead of per-window.
- Softmax denominators: ones-column in v gives row 64 of av;
  reciprocal_approx_fast (not the 8-cyc/elem iterative RECIPROCAL), broadcast
  via a tiny ones-stationary matmul on TensorE, multiply on Pool.
- PSUM: 2 tags x 2 bufs of [128,1024] = exactly 8 banks across all phases.
- All matmul operands bf16; PSUM fp32. Output partials DMA'd as bf16.
"""
import numpy as np
import ml_dtypes

E, H, D, LK = 1024, 16, 64, 128
SCALE = D ** -0.5
B, N = 4, 2048
FG = 512          # features per head-group (8 heads * 64)
NCORES = 8

# banded attention, jc-centric: key chunk jc covers queries [128jc-128, 128jc+256)

_cache = {}


def _bf16(a):
    return np.ascontiguousarray(a.astype(ml_dtypes.bfloat16))


def _build():
    import concourse.bacc as bacc
    import concourse.tile as tile
    import concourse.mybir as mybir
    from concourse.bass import ts

    dt = mybir.dt
    AF = mybir.ActivationFunctionType

    nc = bacc.Bacc("TRN2", target_bir_lowering=False, debug=False,
                   num_devices=NCORES)

    xT = nc.dram_tensor("xT", [E, N], dt.bfloat16, kind="ExternalInput")
    wqT = nc.dram_tensor("wqT", [E, FG], dt.bfloat16, kind="ExternalInput")
    wkT = nc.dram_tensor("wkT", [E, FG], dt.bfloat16, kind="ExternalInput")
    wvT = nc.dram_tensor("wvT", [E, FG], dt.bfloat16, kind="ExternalInput")
    woT = nc.dram_tensor("woT", [FG, E], dt.bfloat16, kind="ExternalInput")
    bqc = nc.dram_tensor("bqc", [128, 4], dt.float32, kind="ExternalInput")
    lmask = nc.dram_tensor("lmask", [128, 384], dt.bfloat16, kind="ExternalInput")
    flagt = nc.dram_tensor("flagt", [128, 2], dt.float32, kind="ExternalInput")
    out = nc.dram_tensor("out", [N, E], dt.bfloat16, kind="ExternalOutput")

    with tile.TileContext(nc) as tc:
        with (
            tc.tile_pool(name="wts", bufs=1) as wts,
            tc.tile_pool(name="xp", bufs=1) as xp,
            tc.tile_pool(name="qkv", bufs=1) as qkv,
            tc.tile_pool(name="att", bufs=4) as att,
            tc.tile_pool(name="small", bufs=4) as small,
            tc.tile_pool(name="obuf", bufs=4) as obuf,
            tc.tile_pool(name="psc", bufs=4, space="PSUM") as psc,
            tc.tile_pool(name="pav", bufs=2, space="PSUM") as pav,
        ):
            # ---- load weights/x/masks ----
            xT_t = [xp.tile([128, N], dt.bfloat16, name=f"xT{i}", tag=f"xT{i}") for i in range(8)]
            for ec in range(8):
                nc.sync.dma_start(xT_t[ec][:], xT[ts(ec, 128), :])
            wq_t = [wts.tile([128, FG], dt.bfloat16, name=f"wq{i}", tag=f"wq{i}") for i in range(8)]
            wk_t = [wts.tile([128, FG], dt.bfloat16, name=f"wk{i}", tag=f"wk{i}") for i in range(8)]
            wv_t = [wts.tile([128, FG], dt.bfloat16, name=f"wv{i}", tag=f"wv{i}") for i in range(8)]
            for ec in range(8):
                nc.sync.dma_start(wq_t[ec][:], wqT[ts(ec, 128), :])
                nc.sync.dma_start(wk_t[ec][:], wkT[ts(ec, 128), :])
                nc.sync.dma_start(wv_t[ec][:], wvT[ts(ec, 128), :])
            wo_t = [wts.tile([128, E], dt.bfloat16, name=f"wo{i}", tag=f"wo{i}") for i in range(4)]
            for fc in range(4):
                nc.sync.dma_start(wo_t[fc][:], woT[ts(fc, 128), :])
            bq_t = small.tile([128, 4], dt.float32, name="bq", tag="bq", bufs=1)
            nc.sync.dma_start(bq_t[:], bqc[:, :])
            lm_t = wts.tile([128, 384], dt.bfloat16, name="lm", tag="lm")
            nc.sync.dma_start(lm_t[:], lmask[:, :])
            flag_t = small.tile([128, 2], dt.float32, name="flag", tag="flag", bufs=1)
            nc.sync.dma_start(flag_t[:], flagt[:, :])

            # ---- q/k projections (features on partitions) ----
            qT_sb = [qkv.tile([128, N], dt.bfloat16, name=f"qT{i}", tag=f"qT{i}") for i in range(4)]
            kT_sb = [qkv.tile([128, N], dt.bfloat16, name=f"kT{i}", tag=f"kT{i}") for i in range(4)]
            for dst, w_t, is_q in ((qT_sb, wq_t, True), (kT_sb, wk_t, False)):
                for fc in range(4):
                    pss = [pav.tile([128, 1024], dt.float32, name="pp", tag="av")
                           for _ in range(2)]
                    for ec in range(8):
                        for half in range(2):
                            for s2 in range(2):
                                nc.tensor.matmul(
                                    pss[half][:, ts(s2, 512)],
                                    w_t[ec][:, ts(fc, 128)],
                                    xT_t[ec][:, ts(half * 2 + s2, 512)],
                                    start=(ec == 0), stop=(ec == 7),
                                    skip_group_check=True)
                    for half in range(2):
                        if is_q:
                            nc.scalar.activation(
                                dst[fc][:, ts(half, 1024)], pss[half][:],
                                AF.Identity, bias=bq_t[:, fc:fc + 1], scale=1.0)
                        else:
                            nc.vector.tensor_copy(
                                dst[fc][:, ts(half, 1024)], pss[half][:])

            # ---- v projection (natural layout, per-head 72-col strided, ones col)
            v_sb = [qkv.tile([128, 8 * 72], dt.bfloat16, name=f"v{i}", tag=f"v{i}") for i in range(16)]
            for tcn2 in range(8):
                ps = pav.tile([128, 1024], dt.float32, name="pp", tag="av")
                for ec in range(8):
                    for s2 in range(2):
                        tcn = tcn2 * 2 + s2
                        nc.tensor.matmul(ps[:, ts(s2, 512)],
                                         xT_t[ec][:, ts(tcn, 128)], wv_t[ec][:],
                                         start=(ec == 0), stop=(ec == 7),
                                         skip_group_check=True)
                for s2 in range(2):
                    tcn = tcn2 * 2 + s2
                    src = ps[:, ts(s2, 512)].rearrange("p (h d) -> p h d", h=8)
                    dst = v_sb[tcn][:].rearrange("p (h d) -> p h d", h=8)[:, :, 0:64]
                    nc.vector.tensor_copy(dst, src)
                    ones = v_sb[tcn][:].rearrange("p (h d) -> p h d", h=8)[:, :, 64:65]
                    nc.vector.memset(ones, 1.0)

            outTn = [qkv.tile([128, N], dt.bfloat16, name=f"outTn{i}", tag=f"outTn{i}") for i in range(4)]

            def head_rows(t, h):
                r0 = (h % 2) * 64
                return t[h // 2][r0:r0 + 64, :]

            def vslice(jc, h):
                return v_sb[jc][:, h * 72:h * 72 + 65]

            def normalize(av_region, den_row, out_dst):
                """av_region [64,512] psum, den_row [1,512] psum -> out_dst bf16.

                reciprocal_approx_fast (custom DVE) is broken for
                cross-partition in/out on this runtime, so hop through a
                builtin cross-partition copy to partition 0 first.
                """
                den0 = small.tile([1, 512], dt.float32, name="den0", tag="den0")
                nc.vector.tensor_copy(den0[:], den_row)
                rec = small.tile([1, 512], dt.float32, name="rec", tag="rec")
                nc.vector.reciprocal_approx_fast(rec[:], den0[:])
                rec64 = small.tile([64, 512], dt.float32, name="rec64", tag="rec64")
                nc.gpsimd.partition_broadcast(rec64[:], rec[:])
                nc.vector.tensor_mul(out_dst, av_region, rec64[:])

            # ---- dense (unmasked) path for slot-0 head: correct for hg0's
            # global head; hg1 discards via flag blend after the banded pass.
            h = 0
            qh = head_rows(qT_sb, h)
            kh = head_rows(kT_sb, h)
            av_g = [pav.tile([128, 1024], dt.float32, name=f"avg{i}", tag="av") for i in range(2)]
            for jc in range(16):
                for s in range(4):
                    ps = psc.tile([128, 512], dt.float32, name="sc", tag="sc")
                    nc.tensor.matmul(ps[:], kh[:, ts(jc, 128)],
                                     qh[:, ts(s, 512)], start=True, stop=True)
                    at = att.tile([128, 512], dt.bfloat16, name="at", tag="at")
                    nc.scalar.activation(at[:], ps[:], AF.Exp, scale=float(SCALE))
                    nc.tensor.matmul(
                        av_g[s // 2][0:65, ts(s % 2, 512)], vslice(jc, h),
                        at[:], start=(jc == 0), stop=(jc == 15),
                        skip_group_check=True)
            # dense normalize -> outTn head-0 rows
            for s in range(4):
                normalize(av_g[s // 2][0:64, ts(s % 2, 512)],
                          av_g[s // 2][64:65, ts(s % 2, 512)],
                          head_rows(outTn, 0)[:, ts(s, 512)])

            # ---- banded path, jc-centric: heads 1..7 normally, head 0 blended
            def banded_normalize(h, s, av01, av23):
                avt = av01 if s < 2 else av23
                c0 = (s % 2) * 512
                if h != 0:
                    normalize(avt[0:64, c0:c0 + 512], avt[64:65, c0:c0 + 512],
                              head_rows(outTn, h)[:, ts(s, 512)])
                else:
                    # blend: outTn_h0 = flag*dense + (1-flag)*banded
                    tmp = small.tile([64, 512], dt.bfloat16, name="tmp", tag="tmp")
                    normalize(avt[0:64, c0:c0 + 512], avt[64:65, c0:c0 + 512],
                              tmp[:])
                    od = head_rows(outTn, 0)[:, ts(s, 512)]
                    nc.vector.tensor_scalar_mul(od, od, flag_t[0:64, 0:1])
                    nc.vector.scalar_tensor_tensor(
                        od, tmp[:], flag_t[0:64, 1:2], od,
                        op0=mybir.AluOpType.mult, op1=mybir.AluOpType.add)

            for h in list(range(1, 8)) + [0]:
                qh = head_rows(qT_sb, h)
                kh = head_rows(kT_sb, h)
                av01 = pav.tile([128, 1024], dt.float32, name="av01", tag="av")
                av23 = pav.tile([128, 1024], dt.float32, name="av23", tag="av")
                for jc in range(16):
                    qs = max(0, 128 * jc - 128)
                    qe = min(N, 128 * jc + 256)
                    w = qe - qs
                    m0 = 128 if jc == 0 else 0
                    sc = psc.tile([128, 512], dt.float32, name="sc", tag="sc")
                    nc.tensor.matmul(sc[:, 0:w], kh[:, ts(jc, 128)],
                                     qh[:, qs:qe], start=True, stop=True)
                    at = att.tile([128, 512], dt.bfloat16, name="at", tag="at")
                    nc.scalar.activation(at[:, 0:w], sc[:, 0:w], AF.Exp,
                                         scale=float(SCALE))
                    if jc % 2 == 0:
                        nc.gpsimd.tensor_mul(at[:, 0:w], at[:, 0:w],
                                             lm_t[:, m0:m0 + w])
                    else:
                        nc.vector.tensor_mul(at[:, 0:w], at[:, 0:w],
                                             lm_t[:, m0:m0 + w])
                    # AV, split at psum bank (512-col) boundaries
                    p = qs
                    while p < qe:
                        p1 = min(qe, (p // 512 + 1) * 512)
                        b = p // 512
                        first = 0 if b == 0 else 4 * b - 1
                        last = 15 if b == 3 else 4 * b + 4
                        dst = av01 if b < 2 else av23
                        c0 = p - (0 if b < 2 else 1024)
                        nc.tensor.matmul(
                            dst[0:65, c0:c0 + (p1 - p)], vslice(jc, h),
                            at[:, p - qs:p1 - qs],
                            start=(jc == first), stop=(jc == last),
                            skip_group_check=True)
                        p = p1
                    # normalize each 512-query slot once its bank is complete
                    if jc == 6:
                        banded_normalize(h, 0, av01, av23)
                    elif jc == 10:
                        banded_normalize(h, 1, av01, av23)
                    elif jc == 14:
                        banded_normalize(h, 2, av01, av23)
                banded_normalize(h, 3, av01, av23)

            # ---- output projection ----
            for tcn in range(16):
                ps = pav.tile([128, 1024], dt.float32, name="pp", tag="av")
                for fc in range(4):
                    for oc in range(2):
                        nc.tensor.matmul(ps[:, ts(oc, 512)],
                                         outTn[fc][:, ts(tcn, 128)],
                                         wo_t[fc][:, ts(oc, 512)],
                                         start=(fc == 0), stop=(fc == 3),
                                         skip_group_check=True)
                ob = obuf.tile([128, 1024], dt.bfloat16, name="ob", tag="ob")
                if tcn % 2 == 0:
                    nc.scalar.copy(ob[:], ps[:])
                else:
                    nc.vector.tensor_copy(ob[:], ps[:])
                nc.sync.dma_start(out[ts(tcn, 128), :], ob[:])
    nc.finalize()
    return nc


def _host_inputs(x, wq, bq, wk, bk, wv, bv, wo, bo):
    """Build the 8 per-core input dicts."""
    r = np.arange(128)[:, None]
    # interior band mask [128, 384]: key chunk jc's window starts at 128jc-128,
    # mask[r, c] = |r + 128 - c| <= 128. jc==0 uses cols [128:384).
    c = np.arange(384)[None, :]
    lm = _bf16((np.abs(r + 128 - c) <= LK).astype(np.float32))

    in_maps = []
    for core in range(NCORES):
        b, hg = core // 2, core % 2
        fsl = slice(hg * FG, (hg + 1) * FG)
        flag = np.zeros((128, 2), np.float32)
        flag[:, 0] = 1.0 if hg == 0 else 0.0
        flag[:, 1] = 1.0 - flag[:, 0]
        in_maps.append({
            "xT": _bf16(x[b].T),
            "wqT": _bf16(wq[fsl].T),
            "wkT": _bf16(wk[fsl].T),
            "wvT": _bf16(wv[fsl].T),
            "woT": _bf16(wo[:, fsl].T),
            "bqc": np.ascontiguousarray(bq[fsl].reshape(4, 128).T, np.float32),
            "lmask": lm,
            "flagt": flag,
        })
    return in_maps


def kernel(x, wq, bq, wk, bk, wv, bv, wo, bo):
    from concourse.bass_utils import run_bass_kernel_spmd

    x, wq, bq, wk, bk, wv, bv, wo, bo = (
        np.asarray(a, np.float32) for a in (x, wq, bq, wk, bk, wv, bv, wo, bo))

    if "nc" not in _cache:
        _cache["nc"] = _build()
    nc = _cache["nc"]

    in_maps = _host_inputs(x, wq, bq, wk, bk, wv, bv, wo, bo)
    res = run_bass_kernel_spmd(nc, in_maps, core_ids=list(range(NCORES)))
    _cache["last_results"] = res

    const = (bo + bv @ wo.T).astype(np.float32)        # [1024]
    out = np.empty((B, N, E), np.float32)
    for b in range(B):
        out[b] = (res.results[2 * b]["out"].astype(np.float32)
                  + res.results[2 * b + 1]["out"].astype(np.float32) + const)
    return out


# revision 29
# speedup vs baseline: 2.0904x; 2.0904x over previous
"""Local/global multihead attention on 8 NeuronCores (Trainium2, Bass/Tile).

Sharding: core c = b*2 + hg  (b = batch 0..3, hg = head-group 0/1, 8 heads each).
Each core: q/k/v projections for its 8 heads on its batch, head-local attention,
out-projection restricted to its head-group columns of wo. Host sums the two
head-group partials per batch and adds bo + bv @ wo.T (softmax rows sum to 1).

Key structure (v2):
- Slot-0 head: computed BOTH ways on every core — dense unmasked 2048-key
  softmax (correct for hg0's global head) AND banded (correct for hg1's local
  head); blended by a per-core flag. No gmask input, no mask work in the dense
  path at all.
- k bias dropped entirely (q.bk is constant per query -> softmax invariant).
- Banded scores for a (head, slot) are packed into two [128,768] psum tiles
  (3 windows each) -> one exp + one mask-mul per tile instead of per-window.
- Softmax denominators: ones-column in v gives row 64 of av;
  reciprocal_approx_fast (not the 8-cyc/elem iterative RECIPROCAL), broadcast
  via a tiny ones-stationary matmul on TensorE, multiply on Pool.
- PSUM: 2 tags x 2 bufs of [128,1024] = exactly 8 banks across all phases.
- All matmul operands bf16; PSUM fp32. Output partials DMA'd as bf16.
"""
import numpy as np
import ml_dtypes

E, H, D, LK = 1024, 16, 64, 128
SCALE = D ** -0.5
B, N = 4, 2048
FG = 512          # features per head-group (8 heads * 64)
NCORES = 8

# banded attention, jc-centric: key chunk jc covers queries [128jc-128, 128jc+256)

_cache = {}


def _bf16(a):
    return np.ascontiguousarray(a.astype(ml_dtypes.bfloat16))


def _build():
    import concourse.bacc as bacc
    import concourse.tile as tile
    import concourse.mybir as mybir
    from concourse.bass import ts

    dt = mybir.dt
    AF = mybir.ActivationFunctionType

    nc = bacc.Bacc("TRN2", target_bir_lowering=False, debug=False,
                   num_devices=NCORES)

    xT = nc.dram_tensor("xT", [E, N], dt.bfloat16, kind="ExternalInput")
    wqT = nc.dram_tensor("wqT", [E, FG], dt.bfloat16, kind="ExternalInput")
    wkT = nc.dram_tensor("wkT", [E, FG], dt.bfloat16, kind="ExternalInput")
    wvT = nc.dram_tensor("wvT", [E, FG], dt.bfloat16, kind="ExternalInput")
    woT = nc.dram_tensor("woT", [FG, E], dt.bfloat16, kind="ExternalInput")
    bqc = nc.dram_tensor("bqc", [128, 4], dt.float32, kind="ExternalInput")
    lmask = nc.dram_tensor("lmask", [128, 384], dt.bfloat16, kind="ExternalInput")
    flagt = nc.dram_tensor("flagt", [128, 2], dt.float32, kind="ExternalInput")
    out = nc.dram_tensor("out", [N, E], dt.bfloat16, kind="ExternalOutput")

    with tile.TileContext(nc) as tc:
        with (
            tc.tile_pool(name="wts", bufs=1) as wts,
            tc.tile_pool(name="xp", bufs=1) as xp,
            tc.tile_pool(name="qkv", bufs=1) as qkv,
            tc.tile_pool(name="att", bufs=6) as att,
            tc.tile_pool(name="small", bufs=4) as small,
            tc.tile_pool(name="obuf", bufs=4) as obuf,
            tc.tile_pool(name="psc", bufs=4, space="PSUM") as psc,
            tc.tile_pool(name="pav", bufs=2, space="PSUM") as pav,
        ):
            # ---- load weights/x/masks ----
            xT_t = [xp.tile([128, N], dt.bfloat16, name=f"xT{i}", tag=f"xT{i}") for i in range(8)]
            for ec in range(8):
                nc.sync.dma_start(xT_t[ec][:], xT[ts(ec, 128), :])
            wq_t = [wts.tile([128, FG], dt.bfloat16, name=f"wq{i}", tag=f"wq{i}") for i in range(8)]
            wk_t = [wts.tile([128, FG], dt.bfloat16, name=f"wk{i}", tag=f"wk{i}") for i in range(8)]
            wv_t = [wts.tile([128, FG], dt.bfloat16, name=f"wv{i}", tag=f"wv{i}") for i in range(8)]
            for ec in range(8):
                nc.sync.dma_start(wq_t[ec][:], wqT[ts(ec, 128), :])
                nc.sync.dma_start(wk_t[ec][:], wkT[ts(ec, 128), :])
                nc.sync.dma_start(wv_t[ec][:], wvT[ts(ec, 128), :])
            wo_t = [wts.tile([128, E], dt.bfloat16, name=f"wo{i}", tag=f"wo{i}") for i in range(4)]
            for fc in range(4):
                nc.sync.dma_start(wo_t[fc][:], woT[ts(fc, 128), :])
            bq_t = small.tile([128, 4], dt.float32, name="bq", tag="bq", bufs=1)
            nc.sync.dma_start(bq_t[:], bqc[:, :])
            lm_t = wts.tile([128, 384], dt.bfloat16, name="lm", tag="lm")
            nc.sync.dma_start(lm_t[:], lmask[:, :])
            flag_t = small.tile([128, 2], dt.float32, name="flag", tag="flag", bufs=1)
            nc.sync.dma_start(flag_t[:], flagt[:, :])

            # ---- q/k projections (features on partitions) ----
            qT_sb = [qkv.tile([128, N], dt.bfloat16, name=f"qT{i}", tag=f"qT{i}") for i in range(4)]
            kT_sb = [qkv.tile([128, N], dt.bfloat16, name=f"kT{i}", tag=f"kT{i}") for i in range(4)]
            for dst, w_t, is_q in ((qT_sb, wq_t, True), (kT_sb, wk_t, False)):
                for fc in range(4):
                    pss = [pav.tile([128, 1024], dt.float32, name="pp", tag="av")
                           for _ in range(2)]
                    for ec in range(8):
                        for half in range(2):
                            for s2 in range(2):
                                nc.tensor.matmul(
                                    pss[half][:, ts(s2, 512)],
                                    w_t[ec][:, ts(fc, 128)],
                                    xT_t[ec][:, ts(half * 2 + s2, 512)],
                                    start=(ec == 0), stop=(ec == 7),
                                    skip_group_check=True)
                    for half in range(2):
                        if is_q:
                            nc.scalar.activation(
                                dst[fc][:, ts(half, 1024)], pss[half][:],
                                AF.Identity, bias=bq_t[:, fc:fc + 1], scale=1.0)
                        else:
                            nc.vector.tensor_copy(
                                dst[fc][:, ts(half, 1024)], pss[half][:])

            # ---- v projection (natural layout, per-head 72-col strided, ones col)
            v_sb = [qkv.tile([128, 8 * 72], dt.bfloat16, name=f"v{i}", tag=f"v{i}") for i in range(16)]
            for tcn2 in range(8):
                ps = pav.tile([128, 1024], dt.float32, name="pp", tag="av")
                for ec in range(8):
                    for s2 in range(2):
                        tcn = tcn2 * 2 + s2
                        nc.tensor.matmul(ps[:, ts(s2, 512)],
                                         xT_t[ec][:, ts(tcn, 128)], wv_t[ec][:],
                                         start=(ec == 0), stop=(ec == 7),
                                         skip_group_check=True)
                for s2 in range(2):
                    tcn = tcn2 * 2 + s2
                    src = ps[:, ts(s2, 512)].rearrange("p (h d) -> p h d", h=8)
                    dst = v_sb[tcn][:].rearrange("p (h d) -> p h d", h=8)[:, :, 0:64]
                    nc.vector.tensor_copy(dst, src)
                    ones = v_sb[tcn][:].rearrange("p (h d) -> p h d", h=8)[:, :, 64:65]
                    nc.vector.memset(ones, 1.0)

            outTn = [qkv.tile([128, N], dt.bfloat16, name=f"outTn{i}", tag=f"outTn{i}") for i in range(4)]

            def head_rows(t, h):
                r0 = (h % 2) * 64
                return t[h // 2][r0:r0 + 64, :]

            def vslice(jc, h):
                return v_sb[jc][:, h * 72:h * 72 + 65]

            def normalize(av_region, den_row, out_dst):
                """av_region [64,512] psum, den_row [1,512] psum -> out_dst bf16.

                reciprocal_approx_fast (custom DVE) is broken for
                cross-partition in/out on this runtime, so hop through a
                builtin cross-partition copy to partition 0 first.
                """
                den0 = small.tile([1, 512], dt.float32, name="den0", tag="den0")
                nc.vector.tensor_copy(den0[:], den_row)
                rec = small.tile([1, 512], dt.float32, name="rec", tag="rec")
                nc.vector.reciprocal_approx_fast(rec[:], den0[:])
                rec64 = small.tile([64, 512], dt.float32, name="rec64", tag="rec64")
                nc.gpsimd.partition_broadcast(rec64[:], rec[:])
                nc.vector.tensor_mul(out_dst, av_region, rec64[:])

            # ---- dense (unmasked) path for slot-0 head: correct for hg0's
            # global head; hg1 discards via flag blend after the banded pass.
            h = 0
            qh = head_rows(qT_sb, h)
            kh = head_rows(kT_sb, h)
            av_g = [pav.tile([128, 1024], dt.float32, name=f"avg{i}", tag="av") for i in range(2)]
            # software-pipelined: AV lags the score/exp stage so the PE queue
            # (strict program order) never waits on the Scalar exp latency.
            LAG = 3
            steps = [(jc, s) for jc in range(16) for s in range(4)]
            atq = {}
            for i in range(len(steps) + LAG):
                if i < len(steps):
                    jc, s = steps[i]
                    ps = psc.tile([128, 512], dt.float32, name="sc", tag="sc")
                    nc.tensor.matmul(ps[:], kh[:, ts(jc, 128)],
                                     qh[:, ts(s, 512)], start=True, stop=True)
                    at = att.tile([128, 512], dt.bfloat16, name="at", tag="at")
                    nc.scalar.activation(at[:], ps[:], AF.Exp, scale=float(SCALE))
                    atq[i] = at
                if i >= LAG:
                    jc, s = steps[i - LAG]
                    nc.tensor.matmul(
                        av_g[s // 2][0:65, ts(s % 2, 512)], vslice(jc, h),
                        atq.pop(i - LAG)[:], start=(jc == 0), stop=(jc == 15),
                        skip_group_check=True)
            # dense normalize -> outTn head-0 rows
            for s in range(4):
                normalize(av_g[s // 2][0:64, ts(s % 2, 512)],
                          av_g[s // 2][64:65, ts(s % 2, 512)],
                          head_rows(outTn, 0)[:, ts(s, 512)])

            # ---- banded path, jc-centric: heads 1..7 normally, head 0 blended
            def banded_normalize(h, s, av01, av23):
                avt = av01 if s < 2 else av23
                c0 = (s % 2) * 512
                if h != 0:
                    normalize(avt[0:64, c0:c0 + 512], avt[64:65, c0:c0 + 512],
                              head_rows(outTn, h)[:, ts(s, 512)])
                else:
                    # blend: outTn_h0 = flag*dense + (1-flag)*banded
                    tmp = small.tile([64, 512], dt.bfloat16, name="tmp", tag="tmp")
                    normalize(avt[0:64, c0:c0 + 512], avt[64:65, c0:c0 + 512],
                              tmp[:])
                    od = head_rows(outTn, 0)[:, ts(s, 512)]
                    nc.vector.tensor_scalar_mul(od, od, flag_t[0:64, 0:1])
                    nc.vector.scalar_tensor_tensor(
                        od, tmp[:], flag_t[0:64, 1:2], od,
                        op0=mybir.AluOpType.mult, op1=mybir.AluOpType.add)

            for h in list(range(1, 8)) + [0]:
                qh = head_rows(qT_sb, h)
                kh = head_rows(kT_sb, h)
                av01 = pav.tile([128, 1024], dt.float32, name="av01", tag="av")
                av23 = pav.tile([128, 1024], dt.float32, name="av23", tag="av")
                atq = {}
                for i in range(16 + LAG):
                    if i < 16:
                        jc = i
                        qs = max(0, 128 * jc - 128)
                        qe = min(N, 128 * jc + 256)
                        w = qe - qs
                        m0 = 128 if jc == 0 else 0
                        sc = psc.tile([128, 512], dt.float32, name="sc", tag="sc")
                        nc.tensor.matmul(sc[:, 0:w], kh[:, ts(jc, 128)],
                                         qh[:, qs:qe], start=True, stop=True)
                        at = att.tile([128, 512], dt.bfloat16, name="at", tag="at")
                        nc.scalar.activation(at[:, 0:w], sc[:, 0:w], AF.Exp,
                                             scale=float(SCALE))
                        nc.vector.tensor_mul(at[:, 0:w], at[:, 0:w],
                                             lm_t[:, m0:m0 + w])
                        atq[i] = (at, qs, qe)
                    if i >= LAG:
                        jc = i - LAG
                        at, qs, qe = atq.pop(jc)
                        # AV, split at psum bank (512-col) boundaries
                        p = qs
                        while p < qe:
                            p1 = min(qe, (p // 512 + 1) * 512)
                            b = p // 512
                            first = 0 if b == 0 else 4 * b - 1
                            last = 15 if b == 3 else 4 * b + 4
                            dst = av01 if b < 2 else av23
                            c0 = p - (0 if b < 2 else 1024)
                            nc.tensor.matmul(
                                dst[0:65, c0:c0 + (p1 - p)], vslice(jc, h),
                                at[:, p - qs:p1 - qs],
                                start=(jc == first), stop=(jc == last),
                                skip_group_check=True)
                            p = p1
                        # normalize each 512-query slot once its bank completes
                        if jc == 6:
                            banded_normalize(h, 0, av01, av23)
                        elif jc == 10:
                            banded_normalize(h, 1, av01, av23)
                        elif jc == 14:
                            banded_normalize(h, 2, av01, av23)
                banded_normalize(h, 3, av01, av23)

            # ---- output projection ----
            for tcn in range(16):
                ps = pav.tile([128, 1024], dt.float32, name="pp", tag="av")
                for fc in range(4):
                    for oc in range(2):
                        nc.tensor.matmul(ps[:, ts(oc, 512)],
                                         outTn[fc][:, ts(tcn, 128)],
                                         wo_t[fc][:, ts(oc, 512)],
                                         start=(fc == 0), stop=(fc == 3),
                                         skip_group_check=True)
                ob = obuf.tile([128, 1024], dt.bfloat16, name="ob", tag="ob")
                if tcn % 2 == 0:
                    nc.scalar.copy(ob[:], ps[:])
                else:
                    nc.vector.tensor_copy(ob[:], ps[:])
                nc.sync.dma_start(out[ts(tcn, 128), :], ob[:])
    nc.finalize()
    return nc


def _host_inputs(x, wq, bq, wk, bk, wv, bv, wo, bo):
    """Build the 8 per-core input dicts."""
    r = np.arange(128)[:, None]
    # interior band mask [128, 384]: key chunk jc's window starts at 128jc-128,
    # mask[r, c] = |r + 128 - c| <= 128. jc==0 uses cols [128:384).
    c = np.arange(384)[None, :]
    lm = _bf16((np.abs(r + 128 - c) <= LK).astype(np.float32))

    in_maps = []
    for core in range(NCORES):
        b, hg = core // 2, core % 2
        fsl = slice(hg * FG, (hg + 1) * FG)
        flag = np.zeros((128, 2), np.float32)
        flag[:, 0] = 1.0 if hg == 0 else 0.0
        flag[:, 1] = 1.0 - flag[:, 0]
        in_maps.append({
            "xT": _bf16(x[b].T),
            "wqT": _bf16(wq[fsl].T),
            "wkT": _bf16(wk[fsl].T),
            "wvT": _bf16(wv[fsl].T),
            "woT": _bf16(wo[:, fsl].T),
            "bqc": np.ascontiguousarray(bq[fsl].reshape(4, 128).T, np.float32),
            "lmask": lm,
            "flagt": flag,
        })
    return in_maps


def kernel(x, wq, bq, wk, bk, wv, bv, wo, bo):
    from concourse.bass_utils import run_bass_kernel_spmd

    x, wq, bq, wk, bk, wv, bv, wo, bo = (
        np.asarray(a, np.float32) for a in (x, wq, bq, wk, bk, wv, bv, wo, bo))

    if "nc" not in _cache:
        _cache["nc"] = _build()
    nc = _cache["nc"]

    in_maps = _host_inputs(x, wq, bq, wk, bk, wv, bv, wo, bo)
    res = run_bass_kernel_spmd(nc, in_maps, core_ids=list(range(NCORES)))
    _cache["last_results"] = res

    const = (bo + bv @ wo.T).astype(np.float32)        # [1024]
    out = np.empty((B, N, E), np.float32)
    for b in range(B):
        out[b] = (res.results[2 * b]["out"].astype(np.float32)
                  + res.results[2 * b + 1]["out"].astype(np.float32) + const)
    return out


# revision 38
# speedup vs baseline: 2.1486x; 1.0278x over previous
"""Local/global multihead attention on 8 NeuronCores (Trainium2, Bass/Tile).

Sharding: core c = b*2 + hg  (b = batch 0..3, hg = head-group 0/1, 8 heads each).
Each core: q/k/v projections for its 8 heads on its batch, head-local attention,
out-projection restricted to its head-group columns of wo. Host sums the two
head-group partials per batch and adds bo + bv @ wo.T (softmax rows sum to 1).

Key structure (v2):
- Slot-0 head: computed BOTH ways on every core — dense unmasked 2048-key
  softmax (correct for hg0's global head) AND banded (correct for hg1's local
  head); blended by a per-core flag. No gmask input, no mask work in the dense
  path at all.
- k bias dropped entirely (q.bk is constant per query -> softmax invariant).
- Banded scores for a (head, slot) are packed into two [128,768] psum tiles
  (3 windows each) -> one exp + one mask-mul per tile instead of per-window.
- Softmax denominators: ones-column in v gives row 64 of av;
  reciprocal_approx_fast (not the 8-cyc/elem iterative RECIPROCAL), broadcast
  via a tiny ones-stationary matmul on TensorE, multiply on Pool.
- PSUM: 2 tags x 2 bufs of [128,1024] = exactly 8 banks across all phases.
- All matmul operands bf16; PSUM fp32. Output partials DMA'd as bf16.
"""
import numpy as np
import ml_dtypes

E, H, D, LK = 1024, 16, 64, 128
SCALE = D ** -0.5
B, N = 4, 2048
FG = 512          # features per head-group (8 heads * 64)
NCORES = 8

# banded attention, jc-centric: key chunk jc covers queries [128jc-128, 128jc+256)

_cache = {}


def _bf16(a):
    return np.ascontiguousarray(a.astype(ml_dtypes.bfloat16))


def _build():
    import concourse.bacc as bacc
    import concourse.tile as tile
    import concourse.mybir as mybir
    from concourse.bass import ts

    dt = mybir.dt
    AF = mybir.ActivationFunctionType

    nc = bacc.Bacc("TRN2", target_bir_lowering=False, debug=False,
                   num_devices=NCORES)

    xT = nc.dram_tensor("xT", [E, N], dt.bfloat16, kind="ExternalInput")
    wqT = nc.dram_tensor("wqT", [E, FG], dt.bfloat16, kind="ExternalInput")
    wkT = nc.dram_tensor("wkT", [E, FG], dt.bfloat16, kind="ExternalInput")
    wvT = nc.dram_tensor("wvT", [E, FG], dt.bfloat16, kind="ExternalInput")
    woT = nc.dram_tensor("woT", [FG, E], dt.bfloat16, kind="ExternalInput")
    bqc = nc.dram_tensor("bqc", [128, 4], dt.float32, kind="ExternalInput")
    lmask = nc.dram_tensor("lmask", [128, 2048], dt.bfloat16, kind="ExternalInput")
    flagt = nc.dram_tensor("flagt", [128, 2], dt.float32, kind="ExternalInput")
    out = nc.dram_tensor("out", [N, E], dt.bfloat16, kind="ExternalOutput")

    with tile.TileContext(nc) as tc:
        with (
            tc.tile_pool(name="wts", bufs=1) as wts,
            tc.tile_pool(name="xp", bufs=1) as xp,
            tc.tile_pool(name="qkv", bufs=1) as qkv,
            tc.tile_pool(name="att", bufs=4) as att,
            tc.tile_pool(name="small", bufs=4) as small,
            tc.tile_pool(name="obuf", bufs=4) as obuf,
            tc.tile_pool(name="psc", bufs=2, space="PSUM") as psc,
            tc.tile_pool(name="pav", bufs=2, space="PSUM") as pav,
        ):
            # ---- load weights/x/masks ----
            xT_t = [xp.tile([128, N], dt.bfloat16, name=f"xT{i}", tag=f"xT{i}") for i in range(8)]
            for ec in range(8):
                nc.sync.dma_start(xT_t[ec][:], xT[ts(ec, 128), :])
            wq_t = [wts.tile([128, FG], dt.bfloat16, name=f"wq{i}", tag=f"wq{i}") for i in range(8)]
            wk_t = [wts.tile([128, FG], dt.bfloat16, name=f"wk{i}", tag=f"wk{i}") for i in range(8)]
            wv_t = [wts.tile([128, FG], dt.bfloat16, name=f"wv{i}", tag=f"wv{i}") for i in range(8)]
            for ec in range(8):
                nc.sync.dma_start(wq_t[ec][:], wqT[ts(ec, 128), :])
                nc.sync.dma_start(wk_t[ec][:], wkT[ts(ec, 128), :])
                nc.sync.dma_start(wv_t[ec][:], wvT[ts(ec, 128), :])
            wo_t = [wts.tile([128, E], dt.bfloat16, name=f"wo{i}", tag=f"wo{i}") for i in range(4)]
            for fc in range(4):
                nc.sync.dma_start(wo_t[fc][:], woT[ts(fc, 128), :])
            bq_t = small.tile([128, 4], dt.float32, name="bq", tag="bq", bufs=1)
            nc.sync.dma_start(bq_t[:], bqc[:, :])
            lm_t = wts.tile([128, 2048], dt.bfloat16, name="lm", tag="lm")
            nc.sync.dma_start(lm_t[:], lmask[:, :])
            flag_t = small.tile([128, 2], dt.float32, name="flag", tag="flag", bufs=1)
            nc.sync.dma_start(flag_t[:], flagt[:, :])
            zt = small.tile([1, 512], dt.bfloat16, name="zt", tag="zt", bufs=1)
            nc.vector.memset(zt[:], 0.0)

            # ---- q/k projections (features on partitions) ----
            qT_sb = [qkv.tile([128, N], dt.bfloat16, name=f"qT{i}", tag=f"qT{i}") for i in range(4)]
            kT_sb = [qkv.tile([128, N], dt.bfloat16, name=f"kT{i}", tag=f"kT{i}") for i in range(4)]
            for dst, w_t, is_q in ((qT_sb, wq_t, True), (kT_sb, wk_t, False)):
                for fc in range(4):
                    pss = [pav.tile([128, 1024], dt.float32, name="pp", tag="av")
                           for _ in range(2)]
                    for ec in range(8):
                        for half in range(2):
                            for s2 in range(2):
                                nc.tensor.matmul(
                                    pss[half][:, ts(s2, 512)],
                                    w_t[ec][:, ts(fc, 128)],
                                    xT_t[ec][:, ts(half * 2 + s2, 512)],
                                    start=(ec == 0), stop=(ec == 7),
                                    skip_group_check=True)
                    for half in range(2):
                        if is_q:
                            nc.scalar.activation(
                                dst[fc][:, ts(half, 1024)], pss[half][:],
                                AF.Identity, bias=bq_t[:, fc:fc + 1], scale=1.0)
                        else:
                            nc.vector.tensor_copy(
                                dst[fc][:, ts(half, 1024)], pss[half][:])

            # ---- v projection (natural layout, per-head 72-col strided, ones col)
            v_sb = [qkv.tile([128, 8 * 72], dt.bfloat16, name=f"v{i}", tag=f"v{i}") for i in range(16)]
            for tcn2 in range(8):
                ps = pav.tile([128, 1024], dt.float32, name="pp", tag="av")
                for ec in range(8):
                    for s2 in range(2):
                        tcn = tcn2 * 2 + s2
                        nc.tensor.matmul(ps[:, ts(s2, 512)],
                                         xT_t[ec][:, ts(tcn, 128)], wv_t[ec][:],
                                         start=(ec == 0), stop=(ec == 7),
                                         skip_group_check=True)
                for s2 in range(2):
                    tcn = tcn2 * 2 + s2
                    src = ps[:, ts(s2, 512)].rearrange("p (h d) -> p h d", h=8)
                    dst = v_sb[tcn][:].rearrange("p (h d) -> p h d", h=8)[:, :, 0:64]
                    nc.vector.tensor_copy(dst, src)
                    ones = v_sb[tcn][:].rearrange("p (h d) -> p h d", h=8)[:, :, 64:65]
                    nc.vector.memset(ones, 1.0)

            outTn = [qkv.tile([128, N], dt.bfloat16, name=f"outTn{i}", tag=f"outTn{i}") for i in range(4)]

            def head_rows(t, h):
                r0 = (h % 2) * 64
                return t[h // 2][r0:r0 + 64, :]

            def vslice(jc, h):
                return v_sb[jc][:, h * 72:h * 72 + 65]

            def normalize(av_region, den_row, out_dst):
                """av_region [64,512] psum, den_row [1,512] psum -> out_dst bf16.

                reciprocal_approx_fast (custom DVE) is broken for
                cross-partition in/out on this runtime, so hop through a
                builtin cross-partition copy to partition 0 first.
                """
                den0 = small.tile([1, 512], dt.float32, name="den0", tag="den0")
                nc.vector.tensor_copy(den0[:], den_row)
                rec = small.tile([1, 512], dt.float32, name="rec", tag="rec")
                nc.vector.reciprocal_approx_fast(rec[:], den0[:])
                rec64 = small.tile([64, 512], dt.float32, name="rec64", tag="rec64")
                nc.gpsimd.partition_broadcast(rec64[:], rec[:])
                nc.vector.tensor_mul(out_dst, av_region, rec64[:])

            # ---- dense (unmasked) path for slot-0 head: correct for hg0's
            # global head; hg1 discards via flag blend after the banded pass.
            h = 0
            qh = head_rows(qT_sb, h)
            kh = head_rows(kT_sb, h)
            av_g = [pav.tile([128, 1024], dt.float32, name=f"avg{i}", tag="av") for i in range(2)]
            # software-pipelined: AV lags the score/exp stage so the PE queue
            # (strict program order) never waits on the Scalar exp latency.
            # s-pairs share one [128,1024] psum tile -> one exp per pair.
            LAG = 2
            steps = [(jc, half) for jc in range(16) for half in range(2)]
            atq = {}
            for i in range(len(steps) + LAG):
                if i < len(steps):
                    jc, half = steps[i]
                    ps = psc.tile([128, 1024], dt.float32, name="sc", tag="sc")
                    for s2 in range(2):
                        nc.tensor.matmul(ps[:, ts(s2, 512)], kh[:, ts(jc, 128)],
                                         qh[:, ts(half * 2 + s2, 512)],
                                         start=True, stop=True,
                                         skip_group_check=True)
                    at = att.tile([128, 1024], dt.bfloat16, name="at", tag="at")
                    nc.scalar.activation(at[:], ps[:], AF.Exp, scale=float(SCALE))
                    atq[i] = at
                if i >= LAG:
                    jc, half = steps[i - LAG]
                    at = atq.pop(i - LAG)
                    for s2 in range(2):
                        nc.tensor.matmul(
                            av_g[half][0:65, ts(s2, 512)], vslice(jc, h),
                            at[:, ts(s2, 512)], start=(jc == 0), stop=(jc == 15),
                            skip_group_check=True)
            # dense normalize -> outTn head-0 rows
            for s in range(4):
                normalize(av_g[s // 2][0:64, ts(s % 2, 512)],
                          av_g[s // 2][64:65, ts(s % 2, 512)],
                          head_rows(outTn, 0)[:, ts(s, 512)])

            # ---- banded path, jc-centric: heads 1..7 normally, head 0 blended
            def banded_normalize(h, s, av01, av23):
                avt = av01 if s < 2 else av23
                c0 = (s % 2) * 512
                if h != 0:
                    normalize(avt[0:64, c0:c0 + 512], avt[64:65, c0:c0 + 512],
                              head_rows(outTn, h)[:, ts(s, 512)])
                else:
                    # blend: outTn_h0 = flag*dense + (1-flag)*banded
                    tmp = small.tile([64, 512], dt.bfloat16, name="tmp", tag="tmp")
                    normalize(avt[0:64, c0:c0 + 512], avt[64:65, c0:c0 + 512],
                              tmp[:])
                    od = head_rows(outTn, 0)[:, ts(s, 512)]
                    nc.vector.tensor_scalar_mul(od, od, flag_t[0:64, 0:1])
                    nc.vector.scalar_tensor_tensor(
                        od, tmp[:], flag_t[0:64, 1:2], od,
                        op0=mybir.AluOpType.mult, op1=mybir.AluOpType.add)

            for h in list(range(1, 8)) + [0]:
                qh = head_rows(qT_sb, h)
                kh = head_rows(kT_sb, h)
                av01 = pav.tile([128, 1024], dt.float32, name="av01", tag="av")
                av23 = pav.tile([128, 1024], dt.float32, name="av23", tag="av")
                atq = {}
                # jc-pairs: even jc's scores at sc[:, 0:w0], odd at [512:512+w1]
                for i in range(8 + LAG):
                    if i < 8:
                        sc = psc.tile([128, 1024], dt.float32, name="sc", tag="sc")
                        wds = []
                        for k in range(2):
                            jc = 2 * i + k
                            qs = max(0, 128 * jc - 128)
                            qe = min(N, 128 * jc + 256)
                            w = qe - qs
                            if k == 0 and w < 512:
                                # zero-fill the gap cols so the paired exp
                                # reads only initialized psum (group start)
                                nc.tensor.matmul(
                                    sc[:, w:512], zt[0:1, 0:128],
                                    zt[0:1, 0:512 - w], start=True, stop=False,
                                    skip_group_check=True)
                                nc.tensor.matmul(
                                    sc[:, 0:w], kh[:, ts(jc, 128)],
                                    qh[:, qs:qe], start=False, stop=True,
                                    skip_group_check=True)
                            else:
                                nc.tensor.matmul(
                                    sc[:, 512 * k:512 * k + w],
                                    kh[:, ts(jc, 128)],
                                    qh[:, qs:qe], start=True, stop=True,
                                    skip_group_check=True)
                            wds.append((qs, qe))
                        at = att.tile([128, 1024], dt.bfloat16, name="at", tag="at")
                        we = 512 + (wds[1][1] - wds[1][0])
                        nc.scalar.activation(at[:, 0:we], sc[:, 0:we], AF.Exp,
                                             scale=float(SCALE))
                        mb = 0 if i == 0 else 1024
                        nc.vector.tensor_mul(at[:, 0:we], at[:, 0:we],
                                             lm_t[:, mb:mb + we])
                        atq[i] = (at, wds)
                    if i >= LAG:
                        at, wds = atq.pop(i - LAG)
                        for k in range(2):
                            jc = 2 * (i - LAG) + k
                            qs, qe = wds[k]
                            # AV, split at psum bank (512-col) boundaries
                            p = qs
                            while p < qe:
                                p1 = min(qe, (p // 512 + 1) * 512)
                                b = p // 512
                                first = 0 if b == 0 else 4 * b - 1
                                last = 15 if b == 3 else 4 * b + 4
                                dst = av01 if b < 2 else av23
                                c0 = p - (0 if b < 2 else 1024)
                                nc.tensor.matmul(
                                    dst[0:65, c0:c0 + (p1 - p)], vslice(jc, h),
                                    at[:, 512 * k + p - qs:512 * k + p1 - qs],
                                    start=(jc == first), stop=(jc == last),
                                    skip_group_check=True)
                                p = p1
                        # normalize each 512-query slot once its bank completes
                        pr = i - LAG
                        if pr == 2:
                            banded_normalize(h, 0, av01, av23)
                        elif pr == 4:
                            banded_normalize(h, 1, av01, av23)
                        elif pr == 6:
                            banded_normalize(h, 2, av01, av23)
                banded_normalize(h, 3, av01, av23)

            # ---- output projection ----
            for tcn in range(16):
                ps = pav.tile([128, 1024], dt.float32, name="pp", tag="av")
                for fc in range(4):
                    for oc in range(2):
                        nc.tensor.matmul(ps[:, ts(oc, 512)],
                                         outTn[fc][:, ts(tcn, 128)],
                                         wo_t[fc][:, ts(oc, 512)],
                                         start=(fc == 0), stop=(fc == 3),
                                         skip_group_check=True)
                ob = obuf.tile([128, 1024], dt.bfloat16, name="ob", tag="ob")
                if tcn % 2 == 0:
                    nc.scalar.copy(ob[:], ps[:])
                else:
                    nc.vector.tensor_copy(ob[:], ps[:])
                nc.sync.dma_start(out[ts(tcn, 128), :], ob[:])
    nc.finalize()
    return nc


def _host_inputs(x, wq, bq, wk, bk, wv, bv, wo, bo):
    """Build the 8 per-core input dicts."""
    r = np.arange(128)[:, None]
    # paired band masks [128, 2048]: cols [0:1024) = jc-pair 0 (shifted even
    # window), cols [1024:2048) = interior pairs. Within a pair tile the even
    # jc's window sits at [0:384) and the odd jc's at [512:896); gaps are 0.
    c = np.arange(384)[None, :]
    interior = (np.abs(r + 128 - c) <= LK).astype(np.float32)
    lm = np.zeros((128, 2048), np.float32)
    lm[:, 0:256] = interior[:, 128:384]        # jc=0 (window clipped at 0)
    lm[:, 512:896] = interior                  # jc=1
    lm[:, 1024:1408] = interior                # even jc >= 2
    lm[:, 1536:1920] = interior                # odd jc
    lm = _bf16(lm)

    in_maps = []
    for core in range(NCORES):
        b, hg = core // 2, core % 2
        fsl = slice(hg * FG, (hg + 1) * FG)
        flag = np.zeros((128, 2), np.float32)
        flag[:, 0] = 1.0 if hg == 0 else 0.0
        flag[:, 1] = 1.0 - flag[:, 0]
        in_maps.append({
            "xT": _bf16(x[b].T),
            "wqT": _bf16(wq[fsl].T),
            "wkT": _bf16(wk[fsl].T),
            "wvT": _bf16(wv[fsl].T),
            "woT": _bf16(wo[:, fsl].T),
            "bqc": np.ascontiguousarray(bq[fsl].reshape(4, 128).T, np.float32),
            "lmask": lm,
            "flagt": flag,
        })
    return in_maps


def kernel(x, wq, bq, wk, bk, wv, bv, wo, bo):
    from concourse.bass_utils import run_bass_kernel_spmd

    x, wq, bq, wk, bk, wv, bv, wo, bo = (
        np.asarray(a, np.float32) for a in (x, wq, bq, wk, bk, wv, bv, wo, bo))

    if "nc" not in _cache:
        _cache["nc"] = _build()
    nc = _cache["nc"]

    in_maps = _host_inputs(x, wq, bq, wk, bk, wv, bv, wo, bo)
    res = run_bass_kernel_spmd(nc, in_maps, core_ids=list(range(NCORES)))
    _cache["last_results"] = res

    const = (bo + bv @ wo.T).astype(np.float32)        # [1024]
    out = np.empty((B, N, E), np.float32)
    for b in range(B):
        out[b] = (res.results[2 * b]["out"].astype(np.float32)
                  + res.results[2 * b + 1]["out"].astype(np.float32) + const)
    return out
